# revision 1
# baseline (speedup 1.0000x reference)
"""Trainium2 Bass kernel for nn_EnhanceDiversityFeatureExtracition.

loss = mean((output - target)^2)
     + ALPHA * sum(G where TAU < G <= 1, off-diagonal)
  G  = cosine Gram of V[f] = conv_w[:, :, f, :].reshape(-1), f in [0, 128)

Device strategy (8 cores, SPMD, no collectives — host reduces):
 - conv_w viewed flat as [65536, 384] (row = (o, i), col = f*3 + k).
   Rows are sharded 8192/core. Each core accumulates the *flat-layout*
   384x384 Gram  G384[c1, c2] = sum_rows W[r, c1] * W[r, c2]  via
   PE matmuls (lhsT = 128-column slice, rhs = full 384 columns) in
   float32r (full-rate fp32 matmul mode, plenty of precision vs. the
   0.035 margin to the TAU threshold).  The true filter Gram is the
   per-k diagonal  S[f1, f2] = sum_k G384[3 f1 + k, 3 f2 + k],
   extracted on the host (384^2 elements — trivial).
 - output/target sharded 1024 rows/core; DVE computes d = a - b and a
   fused (d*1)*d with per-partition accumulate => MSE partial sums.
Host combines partials in float64 and returns the f32 scalar loss.
"""

import numpy as np

ALPHA = 0.0005
TAU = 0.2

P = 128
NCORES = 8

# conv_w [256, 256, 128, 3] -> flat [65536, 384]
W_ROWS = 65536
W_COLS = 384
W_ROWS_PER_CORE = W_ROWS // NCORES  # 8192
W_TILES = 8  # DMA mega-tiles per core
W_J = W_ROWS_PER_CORE // (W_TILES * P)  # 8 chunks per mega-tile

# output/target [8192, 1000]
B_ROWS = 8192
B_COLS = 1000
B_ROWS_PER_CORE = B_ROWS // NCORES  # 1024
M_TILES = 4
M_J = B_ROWS_PER_CORE // (M_TILES * P)  # 2 rows/partition per tile

_CACHE = {}
LAST_RESULTS = None  # BassKernelResults of the most recent run (for test.py)


def _build_nc():
    import concourse.tile as tile
    from concourse import bacc, mybir

    nc = bacc.Bacc("TRN2", target_bir_lowering=False, debug=False,
                   num_devices=NCORES)
    f32 = mybir.dt.float32
    f32r = mybir.dt.float32r

    wsh = nc.dram_tensor("wsh", [W_ROWS_PER_CORE, W_COLS], f32r,
                         kind="ExternalInput").ap()
    osh = nc.dram_tensor("osh", [B_ROWS_PER_CORE, B_COLS], f32,
                         kind="ExternalInput").ap()
    tsh = nc.dram_tensor("tsh", [B_ROWS_PER_CORE, B_COLS], f32,
                         kind="ExternalInput").ap()
    gout = nc.dram_tensor("gout", [P, 3, W_COLS], f32,
                          kind="ExternalOutput").ap()
    mout = nc.dram_tensor("mout", [P, M_TILES], f32,
                          kind="ExternalOutput").ap()

    # row r = t*(P*J) + p*J + j  ->  per-partition contiguous J rows
    wv = wsh.rearrange("(t p j) c -> t p j c", t=W_TILES, p=P)
    ov = osh.rearrange("(t p j) f -> t p j f", t=M_TILES, p=P)
    tv = tsh.rearrange("(t p j) f -> t p j f", t=M_TILES, p=P)

    n_chunks = W_TILES * W_J  # 64 accumulating matmuls per psum tile

    with tile.TileContext(nc) as tc:
        with (
            tc.tile_pool(name="wpool", bufs=3) as wpool,
            tc.tile_pool(name="apool", bufs=2) as apool,
            tc.tile_pool(name="bpool", bufs=2) as bpool,
            tc.tile_pool(name="dpool", bufs=2) as dpool,
            tc.tile_pool(name="acc", bufs=1) as acc,
            tc.tile_pool(name="psum", bufs=1, space="PSUM") as psum,
        ):
            g_ps = [
                psum.tile([P, W_COLS], f32, name=f"g{m}", tag=f"g{m}")
                for m in range(3)
            ]
            mse_cols = acc.tile([P, M_TILES], f32, name="mse_cols")

            def mse_tile(t):
                at = apool.tile([P, M_J, B_COLS], f32, name="at", tag="at")
                bt = bpool.tile([P, M_J, B_COLS], f32, name="bt", tag="bt")
                nc.sync.dma_start(at[:], ov[t])
                nc.sync.dma_start(bt[:], tv[t])
                d = dpool.tile([P, M_J, B_COLS], f32, name="d", tag="d")
                nc.vector.tensor_tensor(d[:], at[:], bt[:],
                                        mybir.AluOpType.subtract)
                d2 = dpool.tile([P, M_J, B_COLS], f32, name="d2", tag="d2")
                nc.vector.scalar_tensor_tensor(
                    d2[:], d[:], 1.0, d[:],
                    op0=mybir.AluOpType.mult, op1=mybir.AluOpType.mult,
                    accum_out=mse_cols[:, t:t + 1],
                )

            for t in range(W_TILES):
                wt = wpool.tile([P, W_J, W_COLS], f32r, name="wt", tag="wt")
                nc.sync.dma_start(wt[:], wv[t])
                for j in range(W_J):
                    chunk = t * W_J + j
                    for m in range(3):
                        nc.tensor.matmul(
                            g_ps[m][:],
                            wt[:, j, m * P:(m + 1) * P],
                            wt[:, j, :],
                            start=(chunk == 0),
                            stop=(chunk == n_chunks - 1),
                        )
                # interleave MSE work through the conv phase
                if t % 2 == 1:
                    mse_tile(t // 2)

            gs = acc.tile([P, 3, W_COLS], f32, name="gs")
            for m in range(3):
                nc.vector.tensor_copy(gs[:, m, :], g_ps[m][:])
            nc.sync.dma_start(gout[:], gs[:])
            nc.sync.dma_start(mout[:], mse_cols[:])

    nc.compile()
    return nc


def kernel(output, target, conv_w):
    global LAST_RESULTS
    from concourse.bass_utils import run_bass_kernel_spmd

    output = np.ascontiguousarray(np.asarray(output, dtype=np.float32))
    target = np.ascontiguousarray(np.asarray(target, dtype=np.float32))
    conv_w = np.ascontiguousarray(np.asarray(conv_w, dtype=np.float32))
    assert output.shape == (B_ROWS, B_COLS)
    assert target.shape == (B_ROWS, B_COLS)
    assert conv_w.shape == (256, 256, 128, 3)

    if "nc" not in _CACHE:
        _CACHE["nc"] = _build_nc()
    nc = _CACHE["nc"]

    w_flat = conv_w.reshape(W_ROWS, W_COLS)
    in_maps = []
    for c in range(NCORES):
        in_maps.append({
            "wsh": w_flat[c * W_ROWS_PER_CORE:(c + 1) * W_ROWS_PER_CORE],
            "osh": output[c * B_ROWS_PER_CORE:(c + 1) * B_ROWS_PER_CORE],
            "tsh": target[c * B_ROWS_PER_CORE:(c + 1) * B_ROWS_PER_CORE],
        })

    res = run_bass_kernel_spmd(nc, in_maps, core_ids=list(range(NCORES)))
    LAST_RESULTS = res

    # ---- host reduction (tiny) ----
    g = np.zeros((P, 3, W_COLS), dtype=np.float64)
    mse_sum = 0.0
    for r in res.results:
        g += r["gout"].astype(np.float64)
        mse_sum += float(r["mout"].astype(np.float64).sum())

    # G384[m*128 + f', c] = g[f', m, c]
    g384 = g.transpose(1, 0, 2).reshape(W_COLS, W_COLS)
    # S[f1, f2] = sum_k G384[3 f1 + k, 3 f2 + k]
    s = np.einsum("ikjk->ij", g384.reshape(P, 3, P, 3))
    norms = np.sqrt(np.diag(s))
    gcos = s / np.outer(norms, norms)
    offdiag = ~np.eye(P, dtype=bool)
    mask = (gcos > TAU) & (gcos <= 1.0) & offdiag
    reg = gcos[mask].sum()

    mse = mse_sum / (B_ROWS * B_COLS)
    return np.array(mse + ALPHA * reg, dtype=np.float32)
